# revision 36
# baseline (speedup 1.0000x reference)
"""MoE layer (top-2 of 8 experts, D=1024, H=2048) on 8 trn2 NeuronCores.

Strategy (expert-parallel, per the sharding hint):
  - Router (tiny: [16384,1024]@[1024,8]) runs on host; its output IS the
    sharding decision ("all-to-all tokens by expert assignment").
  - Core e receives the tokens routed to expert e (padded to a uniform
    capacity C, transposed to [D, C] bf16) plus expert e's weights, and
    computes yT = (relu(w1.T @ x + b1)).T-chain fully on-device:
        stage 1: hT[h, c] = relu(sum_d w1[d,h] * xT[d,c] + b1[h])
        stage 2: yT[d, c] = sum_h w2[h,d] * hT[h,c]
    bf16 matmuls, f32 PSUM accumulation, bf16 output (error budget is
    2e-2; the bf16 store costs ~3e-4 and halves output DMA traffic).
  - Host scatter-adds gate * (y + b2) into the output.

The per-core kernel streams 2048 N=512 matmuls at the ~216ns/MM issue
floor (~94% PE busy).  The remaining time is ramp + drain, so the
emission is organized around them:
  - dummy warm-up matmuls (never read) keep the HAM clock-gate's busy
    window full from the preamble barrier until real data lands, and
    bridge DMA-arrival jitter inside block 0's m-loop,
  - the first x block + first w1 chunks are split across the two HWDGE
    trigger queues in consumption order (m0 first-half, x k-chunks,
    m1..m3), and everything else (bulk weights on the gpsimd SWDGE path,
    x prefetches) is dep-gated behind them so the first-wave transfers
    get the full HBM bandwidth,
  - w1 streams per-m and w2 per-d so dep granularity matches the
    consumption pace (a coarse chunk stalls the PE until its last byte),
  - the final d-tile is split into column halves with the last transfer
    on the otherwise-empty scalar queue, shortening the end-of-kernel
    copy+trigger+transfer chain that the fixed ~8us wrapper teardown
    serializes behind.
"""

import numpy as np
import ml_dtypes

import concourse.bacc as bacc
import concourse.mybir as mybir
import concourse.tile as tile
from concourse.tile_rust import add_dep_helper
from concourse import bass_utils

B, S, D, E, TOPK = 4, 4096, 1024, 8, 2
H = 2 * D
P = 128
KD = D // P    # 8 k-tiles over D
MH = H // P    # 16 h-tiles over H
ND = D // P    # 8 d-tiles over D
FD = 512       # moving free-dim per matmul / column block
N_CORES = 8

BF16 = mybir.dt.bfloat16
F32 = mybir.dt.float32

_cache = {}           # capacity C -> compiled Bacc
last_run_results = None  # BassKernelResults of the most recent device run


def _build(C, with_b1=True):
    """Build + compile the per-core FFN program for token capacity C.

    Weight dram layouts are tile-major (host pre-transposes):
      w1m[m, k, p, j] = w1[k*128+p, m*128+j]   (m-major: stage-1 weight
        column-tile m is a contiguous 32KB chunk per k -> the first
        matmul only needs w1m[0] + the first x block, so compute starts
        ~2us into the kernel instead of after the full 8MB weight load)
      w2d[d, m, p, j] = w2[m*128+p, d*128+j]   (d-major, same idea)
    """
    assert C % P == 0
    nc = bacc.Bacc("TRN2", target_bir_lowering=False, debug=False)
    xT = nc.dram_tensor("xT", [D, C], BF16, kind="ExternalInput").ap()
    w1m = nc.dram_tensor("w1m", [MH, P, KD, P], BF16, kind="ExternalInput").ap()
    w2d = nc.dram_tensor("w2d", [ND, P, MH, P], BF16, kind="ExternalInput").ap()
    b1t = (
        nc.dram_tensor("b1t", [P, MH], F32, kind="ExternalInput").ap()
        if with_b1 else None
    )
    yT = nc.dram_tensor("yT", [D, C], BF16, kind="ExternalOutput").ap()

    blocks = []
    c0 = 0
    while c0 < C:
        fd = min(FD, C - c0)
        blocks.append((c0, fd))
        c0 += fd

    xT_r = xT.rearrange("(k p) c -> p k c", p=P)      # [P, KD, C]
    yT_r = yT.rearrange("(d p) c -> p d c", p=P)      # [P, ND, C]
    w1m_r = w1m.rearrange("m p k j -> p m (k j)")     # [P, MH, KD*P]
    w2d_r = w2d.rearrange("d p m j -> p d (m j)")     # [P, ND, MH*P]

    with tile.TileContext(nc) as tc:
        with (
            tc.tile_pool(name="wpool", bufs=1) as wpool,
            tc.tile_pool(name="xpool", bufs=3) as xpool,
            tc.tile_pool(name="hpool", bufs=3) as hpool,
            tc.tile_pool(name="ypool", bufs=2) as ypool,
            tc.tile_pool(name="ps1", bufs=3, space="PSUM") as ps1,
            tc.tile_pool(name="ps2", bufs=4, space="PSUM") as ps2,
            tc.tile_pool(name="psw", bufs=1, space="PSUM") as psw,
        ):
            # PE warm-up: HAM clock-gates the PE to 1.2 GHz until it sees
            # ~3.4us of sustained busy. Chew on a zeroed tile while the
            # first DMAs are in flight so the real matmuls start at 2.4.
            # mostly-uninitialized SBUF on purpose: the 1-element memset
            # satisfies tile allocation without gating the PE on a real
            # producer, so dummies start the moment the preamble clears
            warm = wpool.tile([P, 2 * P], BF16)
            nc.gpsimd.memset(warm[:1, :1], 0.0)
            wps = psw.tile([P, 2 * P], F32)

            def dummy_mm(count=1):
                # Keeps the PE busy (and the HAM clock-gate warm) while
                # real matmuls wait on DMA; results are never read.
                for _ in range(count):
                    nc.tensor.matmul(
                        wps[:, :2 * P], warm[:, :P], warm[:, :2 * P],
                        start=True, stop=True,
                    )

            dummy_mm(14)

            # First x block + first w1 chunks, split across the two HWDGE
            # trigger queues (sync + scalar) so the ~0.8us trigger
            # latencies overlap and the m=0 k-loop starts as soon as
            # x k0 + w1 m0 land.  Ordering matches consumption pace.
            xb0 = xpool.tile([P, KD, FD], BF16)
            c00, fd0 = blocks[0]
            w1_sb = wpool.tile([P, MH, KD * P], BF16)

            def w1chunk(m):
                return nc.scalar.dma_start(w1_sb[:, m, :], w1m_r[:, m, :])

            # Minimal first wave: x k0 (128KB, sync q) + w1 m0 first half
            # (128KB, scalar q) land ~1.3us after the triggers fire, so the
            # first real matmul starts ~2us earlier than with 256KB chunks.
            # Follow-up chunk sizes match the cold k-loop consumption pace.
            HK = KD // 2 * P
            nc.sync.dma_start(        # sync q: k0, k123
                xb0[:, 0:1, :fd0], xT_r[:, 0:1, c00:c00 + fd0]
            )
            nc.scalar.dma_start(w1_sb[:, 0, :HK], w1m_r[:, 0, :HK])
            nc.sync.dma_start(
                xb0[:, 1:4, :fd0], xT_r[:, 1:4, c00:c00 + fd0]
            )
            nc.scalar.dma_start(w1_sb[:, 0, HK:], w1m_r[:, 0, HK:])
            nc.scalar.dma_start(      # scalar q: m0a, m0b, k4567, m1..m3
                xb0[:, 4:8, :fd0], xT_r[:, 4:8, c00:c00 + fd0]
            )
            w1chunk(1)
            w1chunk(2)
            w1chunk(3)
            if with_b1:
                b1_sb = wpool.tile([P, MH], F32)
                nc.gpsimd.dma_start(b1_sb[:], b1t[:, :])

            # The bulk of the weights streams on the gpsimd SWDGE path.
            # A tiny DVE copy that reads the LAST xb0 chunk gates the bulk
            # stream so it can't starve the first x block of HBM bandwidth.
            gate_sb = wpool.tile([P, 8], BF16)
            # reads the tail of w1 m3 — the last first-wave delivery on the
            # scalar queue (x chunks on the sync queue land even earlier),
            # so the bulk flood can't collide with the m1..m3 stream
            gate_copy = nc.vector.tensor_copy(gate_sb[:1, :8], w1_sb[:1, 3, -8:])
            w2_sb = wpool.tile([P, ND, MH * P], BF16)
            bulk = []
            # Per-m w1 triggers: the stage-1 m-loop consumes one 256KB
            # m-chunk per ~1.7us, so per-m dep granularity keeps the PE
            # fed during the ramp (a 4-m chunk stalls the loop until its
            # LAST m has landed).
            for m in range(4, MH):
                bulk.append(nc.gpsimd.dma_start(
                    w1_sb[:, m, :], w1m_r[:, m, :]
                ))
            # per-d w2 triggers: stage 2's d-loop consumes one 1MB d-chunk
            # per ~3.4us; coarser chunks stall it mid-kernel
            for d in range(ND):
                bulk.append(nc.gpsimd.dma_start(
                    w2_sb[:, d, :], w2d_r[:, d, :]
                ))
            for w in bulk:
                add_dep_helper(
                    w.ins, gate_copy.ins,
                    reason="bulk weights wait for the first wave to land",
                )

            xbs = {}
            gate = {"copy": gate_copy}

            def stage1(blki):
                c0, fd = blocks[blki]
                if blki == 0:
                    xb = xb0
                else:
                    xb = xpool.tile([P, KD, FD], BF16)
                    dma = nc.sync.dma_start(
                        xb[:, :, :fd], xT_r[:, :, c0:c0 + fd]
                    )
                    if blki <= 2:
                        # prefetched blocks must not steal HBM bandwidth
                        # from block 0 (they'd land on a parallel queue)
                        add_dep_helper(
                            dma.ins, gate["copy"].ins,
                            reason="x prefetch waits for x block 0",
                        )
                hT = hpool.tile([P, MH, FD], BF16)
                xbs[blki] = (xb, hT)
                for m in range(MH):
                    ps = ps1.tile([P, FD], F32)
                    for k in range(KD):
                        nc.tensor.matmul(
                            ps[:, :fd],
                            w1_sb[:, m, k * P:(k + 1) * P],
                            xb[:, k, :fd],
                            start=(k == 0),
                            stop=(k == KD - 1),
                        )
                    nc.scalar.activation(
                        hT[:, m, :fd],
                        ps[:, :fd],
                        mybir.ActivationFunctionType.Relu,
                        bias=b1_sb[:, m:m + 1] if with_b1 else 0.0,
                    )
                    if blki == 0 and m < 3:
                        # bridge x-chunk/w1-chunk DMA arrival jitter so the
                        # HAM busy-window stays full through the ramp
                        dummy_mm(2)

            def stage2(blki):
                c0, fd = blocks[blki]
                last = blki == len(blocks) - 1
                _, hT = xbs.pop(blki)
                yb = ypool.tile([P, ND, FD], BF16)
                for d in range(ND):
                    final = last and d == ND - 1
                    if final:
                        # split the very last tile into column halves so
                        # the end-of-kernel copy+DMA chain is half-sized
                        # (the first half drains under the second's MMs)
                        h2 = fd // 2
                        for c1 in (0, h2):
                            ps = ps2.tile([P, FD], F32)
                            for m in range(MH):
                                nc.tensor.matmul(
                                    ps[:, :h2],
                                    w2_sb[:, d, m * P:(m + 1) * P],
                                    hT[:, m, c1:c1 + h2],
                                    start=(m == 0),
                                    stop=(m == MH - 1),
                                )
                            nc.vector.tensor_copy(
                                yb[:, d, c1:c1 + h2], ps[:, :h2]
                            )
                            # the very last transfer gets the empty scalar
                            # HWDGE queue: its trigger fires with no
                            # queueing delay behind earlier y transfers
                            eng = nc.scalar if c1 else nc.sync
                            eng.dma_start(
                                yT_r[:, d, c0 + c1:c0 + c1 + h2],
                                yb[:, d, c1:c1 + h2],
                            )
                        continue
                    ps = ps2.tile([P, FD], F32)
                    for m in range(MH):
                        nc.tensor.matmul(
                            ps[:, :fd],
                            w2_sb[:, d, m * P:(m + 1) * P],
                            hT[:, m, :fd],
                            start=(m == 0),
                            stop=(m == MH - 1),
                        )
                    nc.vector.tensor_copy(yb[:, d, :fd], ps[:, :fd])
                    if last:  # stream the tail out so the final DMA is tiny
                        nc.sync.dma_start(
                            yT_r[:, d, c0:c0 + fd], yb[:, d, :fd]
                        )
                if not last:
                    nc.sync.dma_start(yT_r[:, :, c0:c0 + fd], yb[:, :, :fd])

            # software-pipelined: stage 1 of block b+1 runs (on PE) between
            # stage 1 and stage 2 of block b, hiding the relu-eviction tail
            stage1(0)
            for b in range(len(blocks)):
                if b + 1 < len(blocks):
                    stage1(b + 1)
                stage2(b)

    nc.compile()
    return nc


def _route(x_flat, router_w, router_b):
    """Replicates the reference router in numpy float32."""
    logits = x_flat @ router_w + router_b            # [N, E]
    m = logits.max(axis=-1, keepdims=True)
    p = np.exp(logits - m, dtype=np.float32)
    p /= p.sum(axis=-1, keepdims=True)
    # top-k, ties -> lower index first (matches jax.lax.top_k)
    top_i = np.argsort(-p, axis=-1, kind="stable")[:, :TOPK]
    top_p = np.take_along_axis(p, top_i, axis=-1)
    top_p = top_p / top_p.sum(axis=-1, keepdims=True)
    return top_p, top_i


def kernel(x, router_w, router_b, w1, b1, w2, b2, _trace=False):
    global last_run_results
    x = np.asarray(x, dtype=np.float32)
    router_w = np.asarray(router_w, dtype=np.float32)
    router_b = np.asarray(router_b, dtype=np.float32)
    w1 = np.asarray(w1, dtype=np.float32)
    b1 = np.asarray(b1, dtype=np.float32)
    w2 = np.asarray(w2, dtype=np.float32)
    b2 = np.asarray(b2, dtype=np.float32)

    N = B * S
    x_flat = x.reshape(N, D)
    top_p, top_i = _route(x_flat, router_w, router_b)

    # Tokens per expert (the "all-to-all by expert assignment")
    idx = [np.nonzero((top_i == e).any(axis=-1))[0] for e in range(E)]
    gates = [
        (top_p[idx[e]] * (top_i[idx[e]] == e)).sum(axis=-1) for e in range(E)
    ]
    counts = np.array([len(i) for i in idx])
    # Device capacity: the perfect-balance point (N*K/8, multiple of 512).
    # The few overflow tokens of hotter-than-average experts are handled
    # on the host during the scatter-add (a data-parallel remainder).
    C = max(FD, int(-(-(N * TOPK // N_CORES) // FD) * FD))

    with_b1 = bool(np.any(b1))
    key = (C, with_b1)
    if key not in _cache:
        _cache[key] = _build(C, with_b1=with_b1)
    nc = _cache[key]

    in_maps = []
    for e in range(E):
        n_e = min(int(counts[e]), C)
        xTe = np.zeros((D, C), dtype=ml_dtypes.bfloat16)
        xTe[:, :n_e] = x_flat[idx[e][:n_e]].T
        w1m = np.ascontiguousarray(
            w1[e].reshape(KD, P, MH, P).transpose(2, 1, 0, 3)
        ).astype(ml_dtypes.bfloat16)
        w2d = np.ascontiguousarray(
            w2[e].reshape(MH, P, ND, P).transpose(2, 1, 0, 3)
        ).astype(ml_dtypes.bfloat16)
        im = {
            "xT": xTe,
            "w1m": w1m,
            "w2d": w2d,
        }
        if with_b1:
            im["b1t"] = np.ascontiguousarray(b1[e].reshape(MH, P).T)
        in_maps.append(im)

    res = None
    for attempt in range(3):
        try:
            res = bass_utils.run_bass_kernel_spmd(
                nc, in_maps, list(range(N_CORES)), trace=_trace
            )
            break
        except Exception:
            if attempt == 2:
                raise
    last_run_results = res

    out_flat = np.zeros((N, D), dtype=np.float32)
    for e in range(E):
        n_e = min(int(counts[e]), C)
        y_e = res.results[e]["yT"][:, :n_e].T.astype(np.float32)  # [n_e, D]
        out_flat[idx[e][:n_e]] += gates[e][:n_e, None] * (y_e + b2[e])
        if counts[e] > C:  # host handles the overflow tokens
            hi = idx[e][C:]
            h = np.maximum(x_flat[hi] @ w1[e] + b1[e], 0.0)
            y = h @ w2[e] + b2[e]
            out_flat[hi] += gates[e][C:, None] * y
    return out_flat.reshape(B, S, D)

